# revision 19
# baseline (speedup 1.0000x reference)
"""Trainium2 Bass kernel for nn_G3DCrossAttention (B=2, C=512, L=2048, G=2048, H=8).

Algebraic structure (exact math): exp_p[g,b,:] = exp[b,g]*Wg[:,0]+bg is rank-1, so
k/v collapse to k = e*u_k + c_k, v = e*u_v + c_v.  The j-constant score shift
cancels in softmax, the attention output collapses per head to
    x_attn = w*u_v + c_v,   w_i = f_b(a_i),  a = x_seq @ M + a0,
with f_b(a) = d/da log Z_b(a),  Z_b(a) = sum_j exp(a*e_bj).  The degree-(KD-1)
Chebyshev coefficients of f_b are computed on HOST from the tiny `exp` input
(logZ sampled at 32 nodes, fit + analytic series derivative); the device
evaluates f at all (i,h) via a T_k recurrence in a packed [128,32] layout,
unpacks to [H,T] by one SBUF->SBUF DMA, and applies it as one outer-product
matmul per 128-channel tile.

All weight-only transforms (u_v/c_v, M, a0, b1'=b1+W1@be1, b2''=b2+be1, LN2
folded into Wo'=Wo*g2, bo'=bo+Wo@be2, s2=Wo'@1) are computed on HOST; the
device sees fp16 weights (W1.T, W2.T, Wo'.T), the fp16 seq slice pre-packed to
SBUF layout, and small constant grids.  LN1 apply: x' = g1.(y-mu1).rstd1 (be1
folded into b1'/b2'').  LN2 fold: out = rstd2 .* (Wo'@y2 - s2 x mu2*rstd2...)
done as po = Wo'@y2 + (-s2)(x)q2 per tile, then out = po.*bcast(rstd2) + bo'.

Sharding: data-parallel over L (LC=256 queries/core), full pipeline per core.
DMA: only sync+scalar have HW DGE queues (~140GB/s each); gpsimd is software
DGE.  Queue plan: sync: xs chunks -> repack -> unpack -> half of out;
scalar: consts -> W1; gpsimd: W2 -> Wo -> other half of out.
"""

from contextlib import ExitStack

import numpy as np

import concourse.bass as bass
import concourse.tile as tile
from concourse import bacc, mybir
from concourse.bass_utils import run_bass_kernel_spmd

F32 = mybir.dt.float32
F32R = mybir.dt.float32r
FP16 = mybir.dt.float16
AF = mybir.ActivationFunctionType
OP = mybir.AluOpType

B, C, L, G, H = 2, 512, 2048, 2048, 8
D = C // H
NCORES = 8
LC = L // NCORES              # 256 queries per core
T = B * LC                    # 512 tokens per core, tau = b*LC + l
KC = C // 128                 # 4
KH = (4 * C) // 128           # 16
FP = 32                       # llo width of the packed a/w layout
SCALE = 1.0 / float(np.sqrt(D))
EPS = 1e-5
SCAL = 4.6                    # Chebyshev half-range in a units (|a|max ~ 4.43)
KD = 12                       # Chebyshev series length for f = (logZ)'
MN = 32                       # logZ sample nodes per batch (host)
NWARM = 3                     # PE warm-up matmuls while DMAs land

# ---- smalls grid column layout (f32 [128, SM_NCOL]) -------------------------
SM_CBB = 0                    # [128, KD]  per-batch f coeffs (p//64 = batch)
SM_CV = KD                    # [128, 4]   c_v per kt tile
SM_B1 = KD + 4                # [128, 16]  b1' per mt tile
SM_B2 = KD + 20               # [128, 4]   b2'' per kt
SM_G1 = KD + 24               # [128, 4]   g1 per kt
SM_NS2 = KD + 28              # [128, 4]   -s2 per mt
SM_BO = KD + 32               # [128, 4]   bo' per mt
SM_NCOL = KD + 36

# ---- rows vector layout (f32 [1, RW_NCOL]) ----------------------------------
RW_ONE = 0                    # ones [512]
RW_NS2 = 512                  # -s2 [C]
RW_A0R = 1024                 # a0' [8]
RW_NCOL = 1032

TRACE = False
TRACE_KW = {}
LAST_RESULTS = None
_CACHE = None


def _host_consts():
    """Input-independent matrices for the host Chebyshev fit."""
    m = np.arange(MN)
    theta = np.pi * (2 * m + 1) / (2 * MN)
    xn = (SCAL * np.cos(theta)).astype(np.float64)          # nodes in a units
    F = np.zeros((KD, MN))
    for k in range(KD):
        F[k] = (2.0 / MN) * np.cos(k * theta)
    F[0] *= 0.5
    import numpy.polynomial.chebyshev as Ch
    DER = np.zeros((KD, KD))
    for k in range(KD):
        ck = np.zeros(KD)
        ck[k] = 1
        dd = Ch.chebder(ck)
        DER[:len(dd), k] = dd
    DM = (DER @ F) / SCAL                                   # [KD, MN]
    return xn, DM


_XN, _DM = _host_consts()


def _build():
    nc = bacc.Bacc(debug=False, num_devices=NCORES)

    # seq packed on host to [128, ktp, kt2, b, l] (kt = ktp*2+kt2): 2KB lines
    seqp = nc.dram_tensor("seqp", [128, 2, 2 * B * LC], FP16, kind="ExternalInput")
    smalls = nc.dram_tensor("smalls", [128, SM_NCOL], F32, kind="ExternalInput")
    rowsv = nc.dram_tensor("rowsv", [1, RW_NCOL], F32, kind="ExternalInput")
    uvha = nc.dram_tensor("uvha", [H, C], FP16, kind="ExternalInput")
    m16a = nc.dram_tensor("m16a", [128, KC * 8], FP16, kind="ExternalInput")
    w1a = nc.dram_tensor("w1a", [C, 4 * C], FP16, kind="ExternalInput")   # W1.T
    w2a = nc.dram_tensor("w2a", [4 * C, C], FP16, kind="ExternalInput")   # W2.T
    woa = nc.dram_tensor("woa", [C, C], FP16, kind="ExternalInput")       # Wo'.T
    out_sl = nc.dram_tensor("out_sl", [128, KC, T], FP16, kind="ExternalOutput")

    with tile.TileContext(nc) as tc, ExitStack() as ctx:
        p_w = ctx.enter_context(tc.tile_pool(name="w", bufs=1))
        p_act = ctx.enter_context(tc.tile_pool(name="act", bufs=1))
        p_sm = ctx.enter_context(tc.tile_pool(name="sm", bufs=1))
        ps_mm = ctx.enter_context(tc.tile_pool(name="psmm", bufs=4, space="PSUM"))
        ps_xa = ctx.enter_context(tc.tile_pool(name="psxa", bufs=2, space="PSUM"))
        ps_st = ctx.enter_context(tc.tile_pool(name="psst", bufs=1, space="PSUM"))

        # ---- tiny on-chip constants (no DMA) -----------------------------
        wtile_f = p_sm.tile([128, T], F32, tag="warmf")
        nc.vector.memset(wtile_f[:], 0.0)
        wtile = p_sm.tile([128, T], F32R, tag="warm")
        nc.vector.tensor_copy(wtile[:], wtile_f[:])
        onesk = p_sm.tile([128, 1], FP16, tag="onesk")
        nc.vector.memset(onesk[:], 1.0 / C)
        eps_col = p_sm.tile([1, 1], F32, tag="epsc")
        nc.vector.memset(eps_col[:], EPS)

        # ---- DMA loads ---------------------------------------------------
        # scalar HW queue: small consts then W1
        sm = p_sm.tile([128, SM_NCOL], F32, tag="sm")
        nc.scalar.dma_start(sm[:], smalls[:])
        rows = p_sm.tile([1, RW_NCOL], F32, tag="rows")
        nc.scalar.dma_start(rows[:], rowsv[:])
        uvh = p_sm.tile([H, C], FP16, tag="uvh")
        nc.scalar.dma_start(uvh[:], uvha[:])
        m16 = p_sm.tile([128, KC * 8], FP16, tag="m16")
        nc.scalar.dma_start(m16[:], m16a[:])
        # sync HW queue: xs in 2 pair-chunks (4KB lines)
        xs = p_w.tile([128, KC, B, LC], FP16, tag="xs")
        for ktp in range(2):
            nc.sync.dma_start(
                xs[:, 2 * ktp:2 * ktp + 2, :, :],
                seqp[:, ktp, :].rearrange("p (k b l) -> p k b l", k=2, b=B))
        w1s = p_w.tile([128, KC, 4 * C], FP16, tag="w1")
        nc.scalar.dma_start(w1s[:], w1a.rearrange("(kt p) m -> p kt m", p=128))
        # gpsimd software queue: W2 then Wo
        w2s = p_w.tile([128, KH, C], FP16, tag="w2")
        nc.gpsimd.dma_start(w2s[:], w2a.rearrange("(kh p) m -> p kh m", p=128))
        wos = p_w.tile([128, KC, C], FP16, tag="wo")
        nc.gpsimd.dma_start(wos[:], woa.rearrange("(kt p) m -> p kt m", p=128))

        rowsr = p_sm.tile([1, RW_NCOL], F32R, tag="rowsr")
        nc.vector.tensor_copy(rowsr[:], rows[:])

        # ---- PE warm-up while DMAs land ----------------------------------
        for i in range(NWARM):
            pw = ps_xa.tile([128, T], F32, tag="xa", name=f"warm{i}")
            nc.tensor.matmul(pw[0:8, :], wtile[:, 0:8], wtile[:], start=True, stop=True)

        # ---- a = x_seq @ M' + a0'  (pre-scaled to t units) ---------------
        pa = ps_st.tile([8, T], F32, tag="st", name="pa")
        for kt in range(KC):
            nc.tensor.matmul(pa[:], m16[:, kt * 8:(kt + 1) * 8],
                             xs[:, kt, :, :],
                             start=(kt == 0), stop=False)
        nc.tensor.matmul(pa[:], rowsr[0:1, RW_A0R:RW_A0R + 8],
                         rowsr[0:1, RW_ONE:RW_ONE + T], start=False, stop=True)
        # copy+clamp fused (vector reads PSUM)
        tt_sb = p_sm.tile([8, T], F32, tag="tts")
        nc.vector.tensor_scalar(tt_sb[:], pa[:], -1.0, 1.0, op0=OP.max, op1=OP.min)

        # repack to [128, 32], p = b*64 + h*8 + lhi, free = llo (l=lhi*32+llo)
        tt = p_sm.tile([128, FP], F32, tag="tt")
        for b in range(B):
            nc.sync.dma_start(tt[b * 64:(b + 1) * 64, :],
                              tt_sb[:, b * LC:(b + 1) * LC])

        # ---- Chebyshev T_k recurrence (vector) ---------------------------
        cbb = sm[:, SM_CBB:SM_CBB + KD]
        tt2 = p_sm.tile([128, FP], F32, tag="tt2")
        nc.vector.tensor_add(tt2[:], tt[:], tt[:])
        t_tiles = [None, tt]
        for k in range(2, KD):
            tk = p_sm.tile([128, FP], F32, tag=f"t{k}", name=f"t{k}")
            nc.vector.tensor_mul(tk[:], tt2[:], t_tiles[k - 1][:])
            if k == 2:
                nc.vector.tensor_scalar_sub(tk[:], tk[:], 1.0)   # T0 = 1
            else:
                nc.vector.tensor_sub(tk[:], tk[:], t_tiles[k - 2][:])
            t_tiles.append(tk)
            if k in (5, 9):              # PE keep-warm trickle
                tkr = p_sm.tile([128, 8], F32R, tag="tkr", name=f"tkr{k}")
                nc.gpsimd.tensor_copy(tkr[:], tk[:, 0:8])
                pw = ps_xa.tile([128, T], F32, tag="xa", name=f"trk{k}")
                nc.tensor.matmul(pw[0:8, :], tkr[:], wtile[:], start=True, stop=True)

        # ---- contraction sum_k c_k T_k (vector) --------------------------
        accA = p_sm.tile([128, FP], F32, tag="accA")
        accB = p_sm.tile([128, FP], F32, tag="accB")
        nc.vector.tensor_scalar(accA[:], tt[:], cbb[:, 1:2], cbb[:, 0:1],
                                op0=OP.mult, op1=OP.add)
        cur, nxt = accA, accB
        for k in range(2, KD):
            nc.vector.scalar_tensor_tensor(
                out=nxt[:], in0=t_tiles[k][:], scalar=cbb[:, k:k + 1],
                in1=cur[:], op0=OP.mult, op1=OP.add)
            cur, nxt = nxt, cur
        wp16 = p_sm.tile([128, FP], FP16, tag="wp16")
        nc.vector.tensor_copy(wp16[:], cur[:])

        def trickle(dep, nm):
            tkr = p_sm.tile([128, 8], F32R, tag="tkr", name=f"tkr{nm}")
            nc.gpsimd.tensor_copy(tkr[:], dep[:, 0:8])
            pw = ps_xa.tile([128, T], F32, tag="xa", name=f"trw{nm}")
            nc.tensor.matmul(pw[0:8, :], tkr[:], wtile[:], start=True, stop=True)

        def ln_stats_tile(st2, y_tile, kt, ph):
            """Mean contribution inline; squares on scalar for a deferred pass."""
            st, sqs = st2
            nc.tensor.matmul(st[0:1, :], onesk[:], y_tile[:],
                             start=(kt == 0), stop=(kt == KC - 1))
            sq = p_act.tile([128, T], FP16, tag="sq", bufs=4, name=f"sq{ph}{kt}")
            nc.scalar.activation(sq[:], y_tile[:], AF.Square)
            sqs.append(sq)

        def ln_stats_close(st2):
            st, sqs = st2
            for kt, sq in enumerate(sqs):
                nc.tensor.matmul(st[32:33, :], onesk[:], sq[:],
                                 start=(kt == 0), stop=(kt == KC - 1))

        def ln_stats_open(ph):
            # one PSUM bank holds both accumulators: sum(y) at partition 0,
            # sum(y^2) at partition 32 (both legal matmul out base partitions)
            st = ps_st.tile([33, T], F32, tag="st", name=f"st{ph}")
            return st, []

        # ---- unpack w to [H, T] and apply: y = w*u_v + c_v + x_seq -------
        wH = p_sm.tile([H, T], FP16, tag="wH")
        for b in range(B):
            nc.sync.dma_start(wH[:, b * LC:(b + 1) * LC],
                              wp16[b * 64:(b + 1) * 64, :])
        y_t = []
        st1 = ln_stats_open("a")
        for kt in range(KC):
            xa = ps_xa.tile([128, T], F32, tag="xa", name=f"xa{kt}")
            nc.tensor.matmul(xa[:], uvh[:, kt * 128:(kt + 1) * 128],
                             wH[:], start=True, stop=True)
            yk = p_act.tile([128, T], FP16, tag="y", bufs=4, name=f"y{kt}")
            nc.vector.scalar_tensor_tensor(
                out=yk[:], in0=xa[:], scalar=sm[:, SM_CV + kt:SM_CV + kt + 1],
                in1=xs[:, kt, :, :], op0=OP.add, op1=OP.add)
            y_t.append(yk)
            ln_stats_tile(st1, yk, kt, "a")
            if kt == 1:
                trickle(yk, f"y{kt}")

        def ln_rows(st2, ph, want_mu=False, want_q=False):
            """mean/meansq -> (mu, rstd, q=mu*rstd) rows [1, T]."""
            stA, stB = st2[0][0:1, :], st2[0][32:33, :]
            musq = p_sm.tile([1, T], F32, tag="lnr", bufs=6, name=f"musq{ph}")
            nc.scalar.activation(musq[:], stA, AF.Square)
            var = p_sm.tile([1, T], F32, tag="lnr", bufs=6, name=f"var{ph}")
            nc.vector.tensor_sub(var[:], stB, musq[:])
            std = p_sm.tile([1, T], F32R, tag="lnr", bufs=6, name=f"std{ph}")
            nc.scalar.activation(std[:], var[:], AF.Sqrt, bias=eps_col[:])
            pwln = ps_xa.tile([128, T], F32, tag="xa", name=f"pwln{ph}")
            nc.tensor.matmul(pwln[0:8, :], rowsr[0:1, RW_ONE:RW_ONE + 8], std[:],
                             start=True, stop=True)
            rstd_f = p_sm.tile([1, T], F32, tag="rstdf", bufs=2, name=f"rstdf{ph}")
            nc.vector.reciprocal_approx_fast(rstd_f[:], std[:].bitcast(F32))
            rstd = p_sm.tile([1, T], F32R, tag="rstd", bufs=2, name=f"rstd{ph}")
            nc.vector.tensor_copy(rstd[:], rstd_f[:])
            mu = q = None
            if want_mu:
                mu = p_sm.tile([1, T], F32R, tag="mu", bufs=2, name=f"mu{ph}")
                nc.vector.tensor_copy(mu[:], stA)
            if want_q:
                q = p_sm.tile([1, T], F32R, tag="q", bufs=2, name=f"q{ph}")
                nc.vector.tensor_mul(q[:], stA, rstd_f[:])
            return mu, rstd, q

        # ---- LN1 apply -> x' = g1.(y - mu1).rstd1 ------------------------
        ln_stats_close(st1)
        mu1, rstd1, _ = ln_rows(st1, "a", want_mu=True)
        ones128 = rowsr[0:1, RW_ONE:RW_ONE + 128]
        mu1b = ps_xa.tile([128, T], F32, tag="xa", name="mu1b")
        nc.tensor.matmul(mu1b[:], ones128, mu1[:], start=True, stop=True)
        r1b = ps_xa.tile([128, T], F32, tag="xa", name="r1b")
        nc.tensor.matmul(r1b[:], ones128, rstd1[:], start=True, stop=True)
        x_t = []
        for kt in range(KC):
            yc = p_act.tile([128, T], FP16, tag="tx", bufs=2, name=f"yc{kt}")
            nc.vector.tensor_sub(yc[:], y_t[kt][:], mu1b[:])
            xo = p_act.tile([128, T], FP16, tag="x", bufs=4, name=f"x{kt}")
            nc.vector.scalar_tensor_tensor(
                out=xo[:], in0=yc[:], scalar=sm[:, SM_G1 + kt:SM_G1 + kt + 1],
                in1=r1b[:], op0=OP.mult, op1=OP.mult)
            x_t.append(xo)

        # ---- FFN1: h = relu(W1 @ x' + b1') -------------------------------
        h_t = []
        for mt in range(KH):
            pf = ps_mm.tile([128, T], F32, tag="mm", name=f"pf1{mt}")
            for kt in range(KC):
                nc.tensor.matmul(pf[:], w1s[:, kt, mt * 128:(mt + 1) * 128],
                                 x_t[kt][:], start=(kt == 0), stop=(kt == KC - 1))
            hm = p_act.tile([128, T], FP16, tag="h", bufs=KH, name=f"h{mt}")
            nc.scalar.activation(hm[:], pf[:], AF.Relu,
                                 bias=sm[:, SM_B1 + mt:SM_B1 + mt + 1])
            h_t.append(hm)

        # ---- FFN2 + residual -> y2 = x' + W2@h + b2'' --------------------
        y2_t = []
        st2 = ln_stats_open("b")
        for mt in range(KC):
            pf = ps_mm.tile([128, T], F32, tag="mm", name=f"pf2{mt}")
            for kh in range(KH):
                nc.tensor.matmul(pf[:], w2s[:, kh, mt * 128:(mt + 1) * 128],
                                 h_t[kh][:], start=(kh == 0), stop=(kh == KH - 1))
            y2 = p_act.tile([128, T], FP16, tag="y2", bufs=4, name=f"y2{mt}")
            nc.vector.scalar_tensor_tensor(
                out=y2[:], in0=x_t[mt][:], scalar=sm[:, SM_B2 + mt:SM_B2 + mt + 1],
                in1=pf[:], op0=OP.add, op1=OP.add)
            y2_t.append(y2)
            ln_stats_tile(st2, y2, mt, "b")

        # ---- LN2 folded into output projection ---------------------------
        # out = (Wo'@y2 + (-s2)(x)q2) .* bcast(rstd2) + bo'
        po_t = [ps_mm.tile([128, T], F32, tag="mm", name=f"po{mt}")
                for mt in range(KC)]
        for kt in range(KC):
            for mt in range(KC):
                nc.tensor.matmul(po_t[mt][:],
                                 wos[:, kt, mt * 128:(mt + 1) * 128],
                                 y2_t[kt][:], start=(kt == 0), stop=False)
        ln_stats_close(st2)
        _, rstd2, q2 = ln_rows(st2, "b", want_q=True)
        rb_ps = ps_xa.tile([128, T], F32, tag="xa", name="rb")
        nc.tensor.matmul(rb_ps[:], ones128, rstd2[:], start=True, stop=True)
        rb_sb = p_sm.tile([128, T], F32, tag="rbs")
        nc.vector.tensor_copy(rb_sb[:], rb_ps[:])
        for mt in range(KC):
            nc.tensor.matmul(po_t[mt][:],
                             rowsr[0:1, RW_NS2 + mt * 128:RW_NS2 + (mt + 1) * 128],
                             q2[:], start=False, stop=True)
        for mt in range(KC):
            vm = p_act.tile([128, T], F32, tag="vm", bufs=2, name=f"vm{mt}")
            nc.vector.tensor_mul(vm[:], po_t[mt][:], rb_sb[:])
            om = p_act.tile([128, T], FP16, tag="om", bufs=2, name=f"om{mt}")
            nc.scalar.activation(om[:], vm[:], AF.Identity,
                                 bias=sm[:, SM_BO + mt:SM_BO + mt + 1])
            seng = nc.sync if mt % 2 == 0 else nc.gpsimd
            seng.dma_start(out_sl[:, mt, :], om[:])

    nc.compile()
    return nc


def _host_pack(inputs):
    f32 = lambda x: np.asarray(x, dtype=np.float32)
    Wq, Wk, Wv, Wo = (f32(inputs[k]) for k in ("Wq", "Wk", "Wv", "Wo"))
    W1, W2 = f32(inputs["W1"]), f32(inputs["W2"])
    Wg = f32(inputs["Wg"])[:, 0]
    bg, bq, bv, b1, b2, bo = (f32(inputs[k]) for k in ("bg", "bq", "bv", "b1", "b2", "bo"))
    g1, be1, g2, be2 = (f32(inputs[k]) for k in ("g1", "beta1", "g2", "beta2"))
    expv = np.asarray(inputs["exp"], dtype=np.float64)

    u_k = Wk @ Wg
    u_v = Wv @ Wg
    c_v = Wv @ bg + bv
    M = np.zeros((C, H), np.float32)
    a0 = np.zeros(H, np.float32)
    for h in range(H):
        ukh = u_k[h * D:(h + 1) * D]
        M[:, h] = Wq[h * D:(h + 1) * D, :].T @ ukh
        a0[h] = bq[h * D:(h + 1) * D] @ ukh
    Mp = M * (SCALE / SCAL)
    a0p = a0 * (SCALE / SCAL)
    uvH = np.zeros((H, C), np.float32)
    for h in range(H):
        uvH[h, h * D:(h + 1) * D] = u_v[h * D:(h + 1) * D]
    Wop = Wo * g2[None, :]
    bop = bo + Wo @ be2
    s2 = Wop.sum(1)
    b1p = b1 + W1 @ be1
    b2p = b2 + be1

    # Chebyshev coefficients of f_b = (logZ_b)' from the tiny `exp` input.
    lnz = np.zeros((MN, B))
    for b in range(B):
        lnz[:, b] = np.log(np.exp(_XN[:, None] * expv[b][None, :]).sum(1))
    ck = _DM @ lnz                                           # [KD, B]

    smalls = np.zeros((128, SM_NCOL), np.float32)
    smalls[0:64, SM_CBB:SM_CBB + KD] = ck[:, 0]
    smalls[64:128, SM_CBB:SM_CBB + KD] = ck[:, 1]
    for kt in range(KC):
        smalls[:, SM_CV + kt] = c_v[kt * 128:(kt + 1) * 128]
        smalls[:, SM_B2 + kt] = b2p[kt * 128:(kt + 1) * 128]
        smalls[:, SM_G1 + kt] = g1[kt * 128:(kt + 1) * 128]
        smalls[:, SM_BO + kt] = bop[kt * 128:(kt + 1) * 128]
    for mt in range(KH):
        smalls[:, SM_B1 + mt] = b1p[mt * 128:(mt + 1) * 128]

    rowsv = np.zeros((1, RW_NCOL), np.float32)
    rowsv[0, RW_ONE:RW_ONE + 512] = 1.0
    rowsv[0, RW_NS2:RW_NS2 + C] = -s2
    rowsv[0, RW_A0R:RW_A0R + H] = a0p

    m16 = np.zeros((128, KC * 8), np.float16)
    for kt in range(KC):
        m16[:, kt * 8:(kt + 1) * 8] = Mp[kt * 128:(kt + 1) * 128, :]

    f16t = lambda x: np.ascontiguousarray(x.T, dtype=np.float16)
    return {
        "smalls": smalls,
        "rowsv": rowsv,
        "uvha": np.asarray(uvH, dtype=np.float16),
        "m16a": m16,
        "w1a": f16t(W1),
        "w2a": f16t(W2),
        "woa": f16t(Wop),
    }


def kernel(**inputs):
    global _CACHE, LAST_RESULTS
    if _CACHE is None:
        _CACHE = _build()
    nc = _CACHE

    base = _host_pack(inputs)
    seq = np.asarray(inputs["seq"], dtype=np.float16)
    # pre-pack seq to device layout [128, ktp, kt2, b, l]
    seq4 = seq.reshape(B, 2, 2, 128, L).transpose(3, 1, 2, 0, 4)  # [128,ktp,kt2,B,L]
    in_maps = []
    for c in range(NCORES):
        m = dict(base)
        m["seqp"] = np.ascontiguousarray(
            seq4[:, :, :, :, c * LC:(c + 1) * LC]).reshape(128, 2, 2 * B * LC)
        in_maps.append(m)

    res = run_bass_kernel_spmd(nc, in_maps, list(range(NCORES)), trace=TRACE,
                               **TRACE_KW)
    LAST_RESULTS = res
    out = np.empty((B, C, L), np.float32)
    for c in range(NCORES):
        o = res.results[c]["out_sl"].astype(np.float32)      # [128, KC, B*LC]
        o = o.reshape(128, KC, B, LC).transpose(2, 1, 0, 3)  # [B, KC, 128, LC]
        out[:, :, c * LC:(c + 1) * LC] = o.reshape(B, C, LC)
    return out


# revision 22
# speedup vs baseline: 1.1786x; 1.1786x over previous
"""Trainium2 Bass kernel for nn_G3DCrossAttention (B=2, C=512, L=2048, G=2048, H=8).

Algebraic structure (exact math): exp_p[g,b,:] = exp[b,g]*Wg[:,0]+bg is rank-1, so
k/v collapse to k = e*u_k + c_k, v = e*u_v + c_v.  The j-constant score shift
cancels in softmax, the attention output collapses per head to
    x_attn = w*u_v + c_v,   w_i = f_b(a_i),  a = x_seq @ M + a0,
with f_b(a) = d/da log Z_b(a),  Z_b(a) = sum_j exp(a*e_bj).  The degree-(KD-1)
Chebyshev coefficients of f_b are computed on HOST from the tiny `exp` input
(logZ sampled at 32 nodes, fit + analytic series derivative); the device
evaluates f at all (i,h) via a T_k recurrence in a packed [128,32] layout,
unpacks to [H,T] by one SBUF->SBUF DMA, and applies it as one outer-product
matmul per 128-channel tile.

All weight-only transforms (u_v/c_v, M, a0, b1'=b1+W1@be1, b2''=b2+be1, LN2
folded into Wo'=Wo*g2, bo'=bo+Wo@be2, s2=Wo'@1) are computed on HOST; the
device sees fp16 weights (W1.T, W2.T, Wo'.T), the fp16 seq slice pre-packed to
SBUF layout, and small constant grids.  LN1 apply: x' = g1.(y-mu1).rstd1 (be1
folded into b1'/b2'').  LN2 fold: out = rstd2 .* (Wo'@y2 - s2 x mu2*rstd2...)
done as po = Wo'@y2 + (-s2)(x)q2 per tile, then out = po.*bcast(rstd2) + bo'.

Sharding: data-parallel over L (LC=256 queries/core), full pipeline per core.
DMA: only sync+scalar have HW DGE queues (~140GB/s each); gpsimd is software
DGE.  Queue plan: sync: xs chunks -> repack -> unpack -> half of out;
scalar: consts -> W1; gpsimd: W2 -> Wo -> other half of out.
"""

from contextlib import ExitStack

import numpy as np

import concourse.bass as bass
import concourse.tile as tile
from concourse import bacc, mybir
from concourse.bass_utils import run_bass_kernel_spmd

F32 = mybir.dt.float32
F32R = mybir.dt.float32r
FP16 = mybir.dt.float16
AF = mybir.ActivationFunctionType
OP = mybir.AluOpType

B, C, L, G, H = 2, 512, 2048, 2048, 8
D = C // H
NCORES = 8
LC = L // NCORES              # 256 queries per core
T = B * LC                    # 512 tokens per core, tau = b*LC + l
KC = C // 128                 # 4
KH = (4 * C) // 128           # 16
FP = 32                       # llo width of the packed a/w layout
SCALE = 1.0 / float(np.sqrt(D))
EPS = 1e-5
SCAL = 5.0                    # Chebyshev half-range in a units (|a|max ~ 4.43)
KD = 16                       # Chebyshev series length for f = (logZ)'
MN = 32                       # logZ sample nodes per batch (host)
NWARM = 3                     # PE warm-up matmuls while DMAs land

# ---- smalls grid column layout (f32 [128, SM_NCOL]) -------------------------
SM_CBB = 0                    # [128, KD]  per-batch f coeffs (p//64 = batch)
SM_CV = KD                    # [128, 4]   c_v per kt tile
SM_B1 = KD + 4                # [128, 16]  b1' per mt tile
SM_B2 = KD + 20               # [128, 4]   b2'' per kt
SM_G1 = KD + 24               # [128, 4]   g1 per kt
SM_NS2 = KD + 28              # [128, 4]   -s2 per mt
SM_BO = KD + 32               # [128, 4]   bo' per mt
SM_NCOL = KD + 36

# ---- rows vector layout (f32 [1, RW_NCOL]) ----------------------------------
RW_ONE = 0                    # ones [512]
RW_NS2 = 512                  # -s2 [C]
RW_A0R = 1024                 # a0' [8]
RW_NCOL = 1032

TRACE = False
TRACE_KW = {}
LAST_RESULTS = None
_CACHE = None


def _host_consts():
    """Input-independent matrices for the host Chebyshev fit."""
    m = np.arange(MN)
    theta = np.pi * (2 * m + 1) / (2 * MN)
    xn = (SCAL * np.cos(theta)).astype(np.float64)          # nodes in a units
    F = np.zeros((KD, MN))
    for k in range(KD):
        F[k] = (2.0 / MN) * np.cos(k * theta)
    F[0] *= 0.5
    import numpy.polynomial.chebyshev as Ch
    DER = np.zeros((KD, KD))
    for k in range(KD):
        ck = np.zeros(KD)
        ck[k] = 1
        dd = Ch.chebder(ck)
        DER[:len(dd), k] = dd
    DM = (DER @ F) / SCAL                                   # [KD, MN]
    return xn, DM


_XN, _DM = _host_consts()


def _build():
    nc = bacc.Bacc(debug=False, num_devices=NCORES)

    # seq packed on host to [128, ktp, kt2, b, l] (kt = ktp*2+kt2): 2KB lines
    seqp = nc.dram_tensor("seqp", [128, 2, 2 * B * LC], FP16, kind="ExternalInput")
    smalls = nc.dram_tensor("smalls", [128, SM_NCOL], F32, kind="ExternalInput")
    rowsv = nc.dram_tensor("rowsv", [1, RW_NCOL], F32, kind="ExternalInput")
    uvha = nc.dram_tensor("uvha", [H, C], FP16, kind="ExternalInput")
    m16a = nc.dram_tensor("m16a", [128, KC * 8], FP16, kind="ExternalInput")
    w1a = nc.dram_tensor("w1a", [C, 4 * C], FP16, kind="ExternalInput")   # W1.T
    w2a = nc.dram_tensor("w2a", [4 * C, C], FP16, kind="ExternalInput")   # W2.T
    woa = nc.dram_tensor("woa", [C, C], FP16, kind="ExternalInput")       # Wo'.T
    out_sl = nc.dram_tensor("out_sl", [128, KC, T], FP16, kind="ExternalOutput")

    with tile.TileContext(nc) as tc, ExitStack() as ctx:
        p_w = ctx.enter_context(tc.tile_pool(name="w", bufs=1))
        p_act = ctx.enter_context(tc.tile_pool(name="act", bufs=1))
        p_sm = ctx.enter_context(tc.tile_pool(name="sm", bufs=1))
        ps_mm = ctx.enter_context(tc.tile_pool(name="psmm", bufs=4, space="PSUM"))
        ps_xa = ctx.enter_context(tc.tile_pool(name="psxa", bufs=2, space="PSUM"))
        ps_st = ctx.enter_context(tc.tile_pool(name="psst", bufs=1, space="PSUM"))

        # ---- tiny on-chip constants (no DMA) -----------------------------
        wtile_f = p_sm.tile([128, T], F32, tag="warmf")
        nc.vector.memset(wtile_f[:], 0.0)
        wtile = p_sm.tile([128, T], F32R, tag="warm")
        nc.vector.tensor_copy(wtile[:], wtile_f[:])
        onesk = p_sm.tile([128, 1], FP16, tag="onesk")
        nc.vector.memset(onesk[:], 1.0 / C)
        eps_col = p_sm.tile([1, 1], F32, tag="epsc")
        nc.vector.memset(eps_col[:], EPS)

        # ---- DMA loads ---------------------------------------------------
        # scalar HW queue: small consts (critical-path order) then W1
        rows = p_sm.tile([1, RW_NCOL], F32, tag="rows")
        nc.scalar.dma_start(rows[:], rowsv[:])
        m16 = p_sm.tile([128, KC * 8], FP16, tag="m16")
        nc.scalar.dma_start(m16[:], m16a[:])
        sm = p_sm.tile([128, SM_NCOL], F32, tag="sm")
        nc.scalar.dma_start(sm[:], smalls[:])
        uvh = p_sm.tile([H, C], FP16, tag="uvh")
        nc.scalar.dma_start(uvh[:], uvha[:])
        # sync HW queue: xs in 2 pair-chunks (4KB lines)
        xs = p_w.tile([128, KC, B, LC], FP16, tag="xs")
        for ktp in range(2):
            nc.sync.dma_start(
                xs[:, 2 * ktp:2 * ktp + 2, :, :],
                seqp[:, ktp, :].rearrange("p (k b l) -> p k b l", k=2, b=B))
        w1s = p_w.tile([128, KC, 4 * C], FP16, tag="w1")
        nc.scalar.dma_start(w1s[:], w1a.rearrange("(kt p) m -> p kt m", p=128))
        # gpsimd software queue: W2 then Wo
        w2s = p_w.tile([128, KH, C], FP16, tag="w2")
        nc.gpsimd.dma_start(w2s[:], w2a.rearrange("(kh p) m -> p kh m", p=128))
        wos = p_w.tile([128, KC, C], FP16, tag="wo")
        nc.gpsimd.dma_start(wos[:], woa.rearrange("(kt p) m -> p kt m", p=128))

        rowsr = p_sm.tile([1, RW_NCOL], F32R, tag="rowsr")
        nc.vector.tensor_copy(rowsr[:], rows[:])

        # ---- PE warm-up while DMAs land ----------------------------------
        for i in range(NWARM):
            pw = ps_xa.tile([128, T], F32, tag="xa", name=f"warm{i}")
            nc.tensor.matmul(pw[0:8, :], wtile[:, 0:8], wtile[:], start=True, stop=True)

        # ---- a = x_seq @ M' + a0'  (pre-scaled to t units) ---------------
        pa = ps_st.tile([8, T], F32, tag="st", name="pa")
        for kt in range(KC):
            nc.tensor.matmul(pa[:], m16[:, kt * 8:(kt + 1) * 8],
                             xs[:, kt, :, :],
                             start=(kt == 0), stop=False)
        nc.tensor.matmul(pa[:], rowsr[0:1, RW_A0R:RW_A0R + 8],
                         rowsr[0:1, RW_ONE:RW_ONE + T], start=False, stop=True)
        # copy+clamp fused (vector reads PSUM)
        tt_sb = p_sm.tile([8, T], F32, tag="tts")
        nc.vector.tensor_scalar(tt_sb[:], pa[:], -1.0, 1.0, op0=OP.max, op1=OP.min)

        # repack to [128, 32], p = b*64 + h*8 + lhi, free = llo (l=lhi*32+llo)
        tt = p_sm.tile([128, FP], F32, tag="tt")
        for b in range(B):
            nc.sync.dma_start(tt[b * 64:(b + 1) * 64, :],
                              tt_sb[:, b * LC:(b + 1) * LC])

        # ---- Chebyshev T_k recurrence (vector) ---------------------------
        cbb = sm[:, SM_CBB:SM_CBB + KD]
        tt2 = p_sm.tile([128, FP], F32, tag="tt2")
        nc.vector.tensor_add(tt2[:], tt[:], tt[:])
        t_tiles = [None, tt]
        for k in range(2, KD):
            tk = p_sm.tile([128, FP], F32, tag=f"t{k}", name=f"t{k}")
            nc.vector.tensor_mul(tk[:], tt2[:], t_tiles[k - 1][:])
            if k == 2:
                nc.vector.tensor_scalar_sub(tk[:], tk[:], 1.0)   # T0 = 1
            else:
                nc.vector.tensor_sub(tk[:], tk[:], t_tiles[k - 2][:])
            t_tiles.append(tk)
            if k in (5, 9):              # PE keep-warm trickle
                tkr = p_sm.tile([128, 8], F32R, tag="tkr", name=f"tkr{k}")
                nc.gpsimd.tensor_copy(tkr[:], tk[:, 0:8])
                pw = ps_xa.tile([128, T], F32, tag="xa", name=f"trk{k}")
                nc.tensor.matmul(pw[0:8, :], tkr[:], wtile[:], start=True, stop=True)

        # ---- contraction sum_k c_k T_k (vector) --------------------------
        accA = p_sm.tile([128, FP], F32, tag="accA")
        accB = p_sm.tile([128, FP], F32, tag="accB")
        nc.vector.tensor_scalar(accA[:], tt[:], cbb[:, 1:2], cbb[:, 0:1],
                                op0=OP.mult, op1=OP.add)
        cur, nxt = accA, accB
        for k in range(2, KD):
            nc.vector.scalar_tensor_tensor(
                out=nxt[:], in0=t_tiles[k][:], scalar=cbb[:, k:k + 1],
                in1=cur[:], op0=OP.mult, op1=OP.add)
            cur, nxt = nxt, cur
        wp16 = p_sm.tile([128, FP], FP16, tag="wp16")
        nc.vector.tensor_copy(wp16[:], cur[:])

        def trickle(dep, nm):
            tkr = p_sm.tile([128, 8], F32R, tag="tkr", name=f"tkr{nm}")
            nc.gpsimd.tensor_copy(tkr[:], dep[:, 0:8])
            pw = ps_xa.tile([128, T], F32, tag="xa", name=f"trw{nm}")
            nc.tensor.matmul(pw[0:8, :], tkr[:], wtile[:], start=True, stop=True)

        def ln_stats_tile(st2, y_tile, kt, ph):
            """Mean contribution inline; squares on scalar for a deferred pass."""
            st, sqs = st2
            nc.tensor.matmul(st[0:1, :], onesk[:], y_tile[:],
                             start=(kt == 0), stop=(kt == KC - 1))
            sq = p_act.tile([128, T], FP16, tag="sq", bufs=4, name=f"sq{ph}{kt}")
            nc.scalar.activation(sq[:], y_tile[:], AF.Square)
            sqs.append(sq)

        def ln_stats_close(st2):
            st, sqs = st2
            for kt, sq in enumerate(sqs):
                nc.tensor.matmul(st[32:33, :], onesk[:], sq[:],
                                 start=(kt == 0), stop=(kt == KC - 1))

        def ln_stats_open(ph):
            # one PSUM bank holds both accumulators: sum(y) at partition 0,
            # sum(y^2) at partition 32 (both legal matmul out base partitions)
            st = ps_st.tile([33, T], F32, tag="st", name=f"st{ph}")
            return st, []

        # ---- unpack w to [H, T] and apply: y = w*u_v + c_v + x_seq -------
        wH = p_sm.tile([H, T], FP16, tag="wH")
        for b in range(B):
            nc.sync.dma_start(wH[:, b * LC:(b + 1) * LC],
                              wp16[b * 64:(b + 1) * 64, :])
        y_t = []
        st1 = ln_stats_open("a")
        for kt in range(KC):
            xa = ps_xa.tile([128, T], F32, tag="xa", name=f"xa{kt}")
            nc.tensor.matmul(xa[:], uvh[:, kt * 128:(kt + 1) * 128],
                             wH[:], start=True, stop=True)
            yk = p_act.tile([128, T], FP16, tag="y", bufs=4, name=f"y{kt}")
            nc.vector.scalar_tensor_tensor(
                out=yk[:], in0=xa[:], scalar=sm[:, SM_CV + kt:SM_CV + kt + 1],
                in1=xs[:, kt, :, :], op0=OP.add, op1=OP.add)
            y_t.append(yk)
            ln_stats_tile(st1, yk, kt, "a")
            if kt == 1:
                trickle(yk, f"y{kt}")

        def ln_rows(st2, ph, want_mu=False, want_q=False):
            """mean/meansq -> (mu, rstd, q=mu*rstd) rows [1, T]."""
            stA, stB = st2[0][0:1, :], st2[0][32:33, :]
            musq = p_sm.tile([1, T], F32, tag="lnr", bufs=6, name=f"musq{ph}")
            nc.scalar.activation(musq[:], stA, AF.Square)
            var = p_sm.tile([1, T], F32, tag="lnr", bufs=6, name=f"var{ph}")
            nc.vector.tensor_sub(var[:], stB, musq[:])
            std = p_sm.tile([1, T], F32R, tag="lnr", bufs=6, name=f"std{ph}")
            nc.scalar.activation(std[:], var[:], AF.Sqrt, bias=eps_col[:])
            pwln = ps_xa.tile([128, T], F32, tag="xa", name=f"pwln{ph}")
            nc.tensor.matmul(pwln[0:8, :], rowsr[0:1, RW_ONE:RW_ONE + 8], std[:],
                             start=True, stop=True)
            rstd_f = p_sm.tile([1, T], F32, tag="rstdf", bufs=2, name=f"rstdf{ph}")
            nc.vector.reciprocal_approx_fast(rstd_f[:], std[:].bitcast(F32))
            rstd = p_sm.tile([1, T], F32R, tag="rstd", bufs=2, name=f"rstd{ph}")
            nc.vector.tensor_copy(rstd[:], rstd_f[:])
            mu = q = None
            if want_mu:
                mu = p_sm.tile([1, T], F32R, tag="mu", bufs=2, name=f"mu{ph}")
                nc.vector.tensor_copy(mu[:], stA)
            if want_q:
                q = p_sm.tile([1, T], F32R, tag="q", bufs=2, name=f"q{ph}")
                nc.vector.tensor_mul(q[:], stA, rstd_f[:])
            return mu, rstd, q

        # ---- LN1 apply -> x' = g1.(y - mu1).rstd1 ------------------------
        ln_stats_close(st1)
        mu1, rstd1, _ = ln_rows(st1, "a", want_mu=True)
        ones128 = rowsr[0:1, RW_ONE:RW_ONE + 128]
        mu1b = ps_xa.tile([128, T], F32, tag="xa", name="mu1b")
        nc.tensor.matmul(mu1b[:], ones128, mu1[:], start=True, stop=True)
        r1b = ps_xa.tile([128, T], F32, tag="xa", name="r1b")
        nc.tensor.matmul(r1b[:], ones128, rstd1[:], start=True, stop=True)
        x_t = []
        for kt in range(KC):
            yc = p_act.tile([128, T], FP16, tag="tx", bufs=2, name=f"yc{kt}")
            nc.vector.tensor_sub(yc[:], y_t[kt][:], mu1b[:])
            xo = p_act.tile([128, T], FP16, tag="x", bufs=4, name=f"x{kt}")
            nc.vector.scalar_tensor_tensor(
                out=xo[:], in0=yc[:], scalar=sm[:, SM_G1 + kt:SM_G1 + kt + 1],
                in1=r1b[:], op0=OP.mult, op1=OP.mult)
            x_t.append(xo)

        # ---- FFN1: h = relu(W1 @ x' + b1') -------------------------------
        h_t = []
        for mt in range(KH):
            pf = ps_mm.tile([128, T], F32, tag="mm", name=f"pf1{mt}")
            for kt in range(KC):
                nc.tensor.matmul(pf[:], w1s[:, kt, mt * 128:(mt + 1) * 128],
                                 x_t[kt][:], start=(kt == 0), stop=(kt == KC - 1))
            hm = p_act.tile([128, T], FP16, tag="h", bufs=KH, name=f"h{mt}")
            nc.scalar.activation(hm[:], pf[:], AF.Relu,
                                 bias=sm[:, SM_B1 + mt:SM_B1 + mt + 1])
            h_t.append(hm)

        # ---- FFN2 + residual -> y2 = x' + W2@h + b2'' --------------------
        y2_t = []
        st2 = ln_stats_open("b")
        for mt in range(KC):
            pf = ps_mm.tile([128, T], F32, tag="mm", name=f"pf2{mt}")
            for kh in range(KH):
                nc.tensor.matmul(pf[:], w2s[:, kh, mt * 128:(mt + 1) * 128],
                                 h_t[kh][:], start=(kh == 0), stop=(kh == KH - 1))
            y2 = p_act.tile([128, T], FP16, tag="y2", bufs=4, name=f"y2{mt}")
            nc.vector.scalar_tensor_tensor(
                out=y2[:], in0=x_t[mt][:], scalar=sm[:, SM_B2 + mt:SM_B2 + mt + 1],
                in1=pf[:], op0=OP.add, op1=OP.add)
            y2_t.append(y2)
            ln_stats_tile(st2, y2, mt, "b")

        # ---- LN2 folded into output projection ---------------------------
        # out = (Wo'@y2 + (-s2)(x)q2) .* bcast(rstd2) + bo'
        ln_stats_close(st2)          # first: unblocks the rows chain early
        po_t = [ps_mm.tile([128, T], F32, tag="mm", name=f"po{mt}")
                for mt in range(KC)]
        for kt in range(KC):
            for mt in range(KC):
                nc.tensor.matmul(po_t[mt][:],
                                 wos[:, kt, mt * 128:(mt + 1) * 128],
                                 y2_t[kt][:], start=(kt == 0), stop=False)
        _, rstd2, q2 = ln_rows(st2, "b", want_q=True)
        rb_ps = ps_xa.tile([128, T], F32, tag="xa", name="rb")
        nc.tensor.matmul(rb_ps[:], ones128, rstd2[:], start=True, stop=True)
        rb_sb = p_sm.tile([128, T], F32, tag="rbs")
        nc.vector.tensor_copy(rb_sb[:], rb_ps[:])
        for mt in range(KC):
            nc.tensor.matmul(po_t[mt][:],
                             rowsr[0:1, RW_NS2 + mt * 128:RW_NS2 + (mt + 1) * 128],
                             q2[:], start=False, stop=True)
        om_h = [p_act.tile([128, 2, T], FP16, tag="om", bufs=2, name=f"om{q}")
                for q in range(2)]
        for mt in range(KC):
            vm = p_act.tile([128, T], F32, tag="vm", bufs=2, name=f"vm{mt}")
            nc.vector.tensor_mul(vm[:], po_t[mt][:], rb_sb[:])
            om = om_h[mt // 2]
            nc.scalar.activation(om[:, mt % 2, :], vm[:], AF.Identity,
                                 bias=sm[:, SM_BO + mt:SM_BO + mt + 1])
            if mt % 2 == 1:
                seng = nc.sync if mt == 1 else nc.gpsimd
                seng.dma_start(out_sl[:, mt - 1:mt + 1, :], om[:])

    nc.compile()
    return nc


def _host_pack(inputs):
    f32 = lambda x: np.asarray(x, dtype=np.float32)
    Wq, Wk, Wv, Wo = (f32(inputs[k]) for k in ("Wq", "Wk", "Wv", "Wo"))
    W1, W2 = f32(inputs["W1"]), f32(inputs["W2"])
    Wg = f32(inputs["Wg"])[:, 0]
    bg, bq, bv, b1, b2, bo = (f32(inputs[k]) for k in ("bg", "bq", "bv", "b1", "b2", "bo"))
    g1, be1, g2, be2 = (f32(inputs[k]) for k in ("g1", "beta1", "g2", "beta2"))
    expv = np.asarray(inputs["exp"], dtype=np.float64)

    u_k = Wk @ Wg
    u_v = Wv @ Wg
    c_v = Wv @ bg + bv
    M = np.zeros((C, H), np.float32)
    a0 = np.zeros(H, np.float32)
    for h in range(H):
        ukh = u_k[h * D:(h + 1) * D]
        M[:, h] = Wq[h * D:(h + 1) * D, :].T @ ukh
        a0[h] = bq[h * D:(h + 1) * D] @ ukh
    Mp = M * (SCALE / SCAL)
    a0p = a0 * (SCALE / SCAL)
    uvH = np.zeros((H, C), np.float32)
    for h in range(H):
        uvH[h, h * D:(h + 1) * D] = u_v[h * D:(h + 1) * D]
    Wop = Wo * g2[None, :]
    bop = bo + Wo @ be2
    s2 = Wop.sum(1)
    b1p = b1 + W1 @ be1
    b2p = b2 + be1

    # Chebyshev coefficients of f_b = (logZ_b)' from the tiny `exp` input.
    lnz = np.zeros((MN, B))
    for b in range(B):
        lnz[:, b] = np.log(np.exp(_XN[:, None] * expv[b][None, :]).sum(1))
    ck = _DM @ lnz                                           # [KD, B]

    smalls = np.zeros((128, SM_NCOL), np.float32)
    smalls[0:64, SM_CBB:SM_CBB + KD] = ck[:, 0]
    smalls[64:128, SM_CBB:SM_CBB + KD] = ck[:, 1]
    for kt in range(KC):
        smalls[:, SM_CV + kt] = c_v[kt * 128:(kt + 1) * 128]
        smalls[:, SM_B2 + kt] = b2p[kt * 128:(kt + 1) * 128]
        smalls[:, SM_G1 + kt] = g1[kt * 128:(kt + 1) * 128]
        smalls[:, SM_BO + kt] = bop[kt * 128:(kt + 1) * 128]
    for mt in range(KH):
        smalls[:, SM_B1 + mt] = b1p[mt * 128:(mt + 1) * 128]

    rowsv = np.zeros((1, RW_NCOL), np.float32)
    rowsv[0, RW_ONE:RW_ONE + 512] = 1.0
    rowsv[0, RW_NS2:RW_NS2 + C] = -s2
    rowsv[0, RW_A0R:RW_A0R + H] = a0p

    m16 = np.zeros((128, KC * 8), np.float16)
    for kt in range(KC):
        m16[:, kt * 8:(kt + 1) * 8] = Mp[kt * 128:(kt + 1) * 128, :]

    f16t = lambda x: np.ascontiguousarray(x.T, dtype=np.float16)
    return {
        "smalls": smalls,
        "rowsv": rowsv,
        "uvha": np.asarray(uvH, dtype=np.float16),
        "m16a": m16,
        "w1a": f16t(W1),
        "w2a": f16t(W2),
        "woa": f16t(Wop),
    }


def kernel(**inputs):
    global _CACHE, LAST_RESULTS
    if _CACHE is None:
        _CACHE = _build()
    nc = _CACHE

    base = _host_pack(inputs)
    seq = np.asarray(inputs["seq"], dtype=np.float16)
    # pre-pack seq to device layout [128, ktp, kt2, b, l]
    seq4 = seq.reshape(B, 2, 2, 128, L).transpose(3, 1, 2, 0, 4)  # [128,ktp,kt2,B,L]
    in_maps = []
    for c in range(NCORES):
        m = dict(base)
        m["seqp"] = np.ascontiguousarray(
            seq4[:, :, :, :, c * LC:(c + 1) * LC]).reshape(128, 2, 2 * B * LC)
        in_maps.append(m)

    res = run_bass_kernel_spmd(nc, in_maps, list(range(NCORES)), trace=TRACE,
                               **TRACE_KW)
    LAST_RESULTS = res
    out = np.empty((B, C, L), np.float32)
    for c in range(NCORES):
        o = res.results[c]["out_sl"].astype(np.float32)      # [128, KC, B*LC]
        o = o.reshape(128, KC, B, LC).transpose(2, 1, 0, 3)  # [B, KC, 128, LC]
        out[:, :, c * LC:(c + 1) * LC] = o.reshape(B, C, LC)
    return out


# revision 25
# speedup vs baseline: 1.2278x; 1.0417x over previous
"""Trainium2 Bass kernel for nn_G3DCrossAttention (B=2, C=512, L=2048, G=2048, H=8).

Algebraic structure (exact math): exp_p[g,b,:] = exp[b,g]*Wg[:,0]+bg is rank-1, so
k/v collapse to k = e*u_k + c_k, v = e*u_v + c_v.  The j-constant score shift
cancels in softmax, the attention output collapses per head to
    x_attn = w*u_v + c_v,   w_i = f_b(a_i),  a = x_seq @ M + a0,
with f_b(a) = d/da log Z_b(a),  Z_b(a) = sum_j exp(a*e_bj).  f_b is fitted on
HOST from the tiny `exp` input (logZ at 32 Chebyshev nodes -> series derivative
-> degree-11 monomial coefficients); the device evaluates f via Estrin's scheme
in a packed [128,32] layout (t^2/t^4 on the scalar engine, 16 vector ops),
unpacks to [H,T] by one SBUF->SBUF DMA, and applies it as one outer-product
matmul per 128-channel tile.

LN1 statistics are decomposed so the expensive part runs in the DMA window:
with y = y0 + uv.w (y0 = x_seq + c_v, rank-8 head structure), sum_c y and
sum_c y^2 reduce to xs-only matmuls (early) plus tiny wH-corrections:
  sum y   = sum y0 + sum_h uvs_h w_h
  sum y^2 = sum y0^2 + sum_h w_h (2 P_h + uvsq_h w_h),  P = uvT-blockdiag @ y0
All weight-only transforms (u_v/c_v, M, a0, b1'=b1+W1@be1, b2''=b2+be1, LN2
folded into Wo'=Wo*g2, bo'=bo+Wo@be2, s2=Wo'@1) are computed on HOST.
LN1 apply: x' = g1.(y-mu1).rstd1.  LN2 fold: po = Wo'@y2 + (-s2)(x)mu2, then
out = po .* bcast(rstd2) + bo'.

Sharding: data-parallel over L (LC=256 queries/core), full pipeline per core.
DMA: only sync+scalar have HW DGE queues (~140GB/s each; ~2us latency per
dma_start, so small constants ride ONE consolidated grid).  Queue plan:
sync: xs chunks -> repack -> unpack -> out half; scalar: grid, uvh, W1, rows
-> out half; gpsimd (software DGE): W2 -> Wo.
"""

from contextlib import ExitStack

import numpy as np

import concourse.bass as bass
import concourse.tile as tile
from concourse import bacc, mybir
from concourse.bass_utils import run_bass_kernel_spmd

F32 = mybir.dt.float32
F32R = mybir.dt.float32r
FP16 = mybir.dt.float16
AF = mybir.ActivationFunctionType
OP = mybir.AluOpType

B, C, L, G, H = 2, 512, 2048, 2048, 8
D = C // H
NCORES = 8
LC = L // NCORES              # 256 queries per core
T = B * LC                    # 512 tokens per core, tau = b*LC + l
KC = C // 128                 # 4
KH = (4 * C) // 128           # 16
FP = 32                       # llo width of the packed a/w layout
SCALE = 1.0 / float(np.sqrt(D))
EPS = 1e-5
SCAL = 4.6                    # fit half-range in a units (|a|max ~ 4.43)
KD = 12                       # series length for f = (logZ)'
MN = 32                       # logZ sample nodes per batch (host)
NWARM = 3                     # PE warm-up matmuls while DMAs land

# ---- const grid column layout (f32 [128, GN]) -------------------------------
G_CBB = 0                     # [128, KD] monomial d coeffs (p//64 = batch)
G_CV = KD                     # [128, 4]   c_v per kt
G_B1 = KD + 4                 # [128, 16]  b1' per mt
G_B2 = KD + 20                # [128, 4]   b2'' per kt
G_G1 = KD + 24                # [128, 4]   g1 per kt
G_BO = KD + 28                # [128, 4]   bo' per mt
G_A0 = KD + 32                # [8, 1]     a0' col
G_CVS = KD + 33               # [1, 1]     sum(c_v)/C
G_PC = KD + 34                # [8, 1]     sum_{c in h} uv_c cv_c
G_UVQ2 = KD + 35              # [8, 1]     sum_{c in h} uv_c^2 / 2
G_F16 = KD + 36               # [128, 2]f32 = [.,4]fp16: uvs/C | 2/C
G_UVT = KD + 38               # [128,16]f32 = [.,32]fp16: uvT blockdiag cols
G_M16 = KD + 54               # [128,16]f32 = [.,32]fp16: M' cols
GN = KD + 70

RW_NS2 = 0                    # rows: -s2 [C] (tail only)
RW_NCOL = 512

TRACE = False
TRACE_KW = {}
LAST_RESULTS = None
_CACHE = None


def _host_consts():
    """Input-independent matrices for the host fit."""
    m = np.arange(MN)
    theta = np.pi * (2 * m + 1) / (2 * MN)
    xn = (SCAL * np.cos(theta)).astype(np.float64)          # nodes in a units
    F = np.zeros((KD, MN))
    for k in range(KD):
        F[k] = (2.0 / MN) * np.cos(k * theta)
    F[0] *= 0.5
    import numpy.polynomial.chebyshev as Ch
    DER = np.zeros((KD, KD))
    for k in range(KD):
        ck = np.zeros(KD)
        ck[k] = 1
        dd = Ch.chebder(ck)
        DER[:len(dd), k] = dd
    DM = (DER @ F) / SCAL                                   # [KD, MN]
    return xn, DM


_XN, _DM = _host_consts()


def _build():
    nc = bacc.Bacc(debug=False, num_devices=NCORES)

    # seq packed on host to [128, ktp, kt2, b, l] (kt = ktp*2+kt2): 2KB lines
    seqp = nc.dram_tensor("seqp", [128, 2, 2 * B * LC], FP16, kind="ExternalInput")
    grid = nc.dram_tensor("grid", [128, GN], F32, kind="ExternalInput")
    rowsv = nc.dram_tensor("rowsv", [1, RW_NCOL], F32, kind="ExternalInput")
    uvha = nc.dram_tensor("uvha", [H, C], FP16, kind="ExternalInput")
    w1a = nc.dram_tensor("w1a", [C, 4 * C], FP16, kind="ExternalInput")   # W1.T
    w2a = nc.dram_tensor("w2a", [4 * C, C], FP16, kind="ExternalInput")   # W2.T
    woa = nc.dram_tensor("woa", [C, C], FP16, kind="ExternalInput")       # Wo'.T
    out_sl = nc.dram_tensor("out_sl", [128, KC, T], FP16, kind="ExternalOutput")

    with tile.TileContext(nc) as tc, ExitStack() as ctx:
        p_w = ctx.enter_context(tc.tile_pool(name="w", bufs=1))
        p_act = ctx.enter_context(tc.tile_pool(name="act", bufs=1))
        p_sm = ctx.enter_context(tc.tile_pool(name="sm", bufs=1))
        ps_mm = ctx.enter_context(tc.tile_pool(name="psmm", bufs=4, space="PSUM"))
        ps_xa = ctx.enter_context(tc.tile_pool(name="psxa", bufs=2, space="PSUM"))
        ps_st = ctx.enter_context(tc.tile_pool(name="psst", bufs=1, space="PSUM"))

        # ---- tiny on-chip constants (no DMA) -----------------------------
        wtile_f = p_sm.tile([128, T], F32, tag="warmf")
        nc.vector.memset(wtile_f[:], 0.0)
        wtile = p_sm.tile([128, T], F32R, tag="warm")
        nc.vector.tensor_copy(wtile[:], wtile_f[:])
        onesk = p_sm.tile([128, 1], FP16, tag="onesk")
        nc.vector.memset(onesk[:], 1.0 / C)
        onesf = p_sm.tile([1, 128], F32, tag="onesf")
        nc.vector.memset(onesf[:], 1.0)
        ones128 = p_sm.tile([1, 128], F32R, tag="ones128")
        nc.vector.tensor_copy(ones128[:], onesf[:])
        eps_col = p_sm.tile([1, 1], F32, tag="epsc")
        nc.vector.memset(eps_col[:], EPS)

        # ---- DMA loads ---------------------------------------------------
        # scalar HW queue: const grid, uvh, W1, rows
        gr = p_sm.tile([128, GN], F32, tag="gr")
        nc.scalar.dma_start(gr[:], grid[:])
        uvh = p_sm.tile([72, C], FP16, tag="uvh")
        nc.scalar.dma_start(uvh[64:72, :], uvha[:])
        w1s = p_w.tile([128, KC, 4 * C], FP16, tag="w1")
        nc.scalar.dma_start(w1s[:], w1a.rearrange("(kt p) m -> p kt m", p=128))
        rows = p_sm.tile([1, RW_NCOL], F32, tag="rows")
        nc.scalar.dma_start(rows[:], rowsv[:])
        # sync HW queue: xs in 2 pair-chunks (4KB lines)
        xs = p_w.tile([128, KC, B, LC], FP16, tag="xs")
        for ktp in range(2):
            nc.sync.dma_start(
                xs[:, 2 * ktp:2 * ktp + 2, :, :],
                seqp[:, ktp, :].rearrange("p (k b l) -> p k b l", k=2, b=B))
        # gpsimd software queue: W2 then Wo
        w2s = p_w.tile([128, KH, C], FP16, tag="w2")
        nc.gpsimd.dma_start(w2s[:], w2a.rearrange("(kh p) m -> p kh m", p=128))
        wos = p_w.tile([128, KC, C], FP16, tag="wo")
        nc.gpsimd.dma_start(wos[:], woa.rearrange("(kt p) m -> p kt m", p=128))

        m16 = gr[:, G_M16:G_M16 + 16].bitcast(FP16)          # [128, 32]
        uvt16 = gr[:, G_UVT:G_UVT + 16].bitcast(FP16)        # [128, 32]
        f16v = gr[:, G_F16:G_F16 + 2].bitcast(FP16)          # [128, 4]
        uvs_col = f16v[64:72, 0:1]
        twoC_col = f16v[64:72, 1:2]

        rowsr = p_sm.tile([1, RW_NCOL], F32R, tag="rowsr")
        nc.vector.tensor_copy(rowsr[:], rows[:])

        # ---- PE warm-up while DMAs land ----------------------------------
        for i in range(NWARM):
            pw = ps_xa.tile([128, T], F32, tag="xa", name=f"warm{i}")
            nc.tensor.matmul(pw[0:8, :], wtile[:, 0:8], wtile[:], start=True, stop=True)

        # ---- a = x_seq @ M' (pre-scaled to t units); a0 added in the copy
        pa = ps_st.tile([72, T], F32, tag="st", name="pa")
        for kt in range(KC):
            nc.tensor.matmul(pa[64:72, :], m16[:, kt * 8:(kt + 1) * 8],
                             xs[:, kt, :, :],
                             start=(kt == 0), stop=(kt == KC - 1))
        tt_sb = p_sm.tile([72, T], F32, tag="tts")
        nc.scalar.activation(tt_sb[64:72, :], pa[64:72, :], AF.Identity,
                             bias=gr[64:72, G_A0:G_A0 + 1])

        # ---- LN1 stats, xs-only part (runs in the DMA window) ------------
        # st1: sum(y)/C at p0, sum(y^2)/C at p32, P = uvT@y0 at p64-71
        st1 = ps_st.tile([72, T], F32, tag="st", name="st1")
        for kt in range(KC):
            nc.tensor.matmul(st1[0:1, :], onesk[:], xs[:, kt, :, :],
                             start=(kt == 0), stop=False)
        for kt in range(KC):
            nc.tensor.matmul(st1[64:72, :], uvt16[:, kt * 8:(kt + 1) * 8],
                             xs[:, kt, :, :], start=(kt == 0), stop=(kt == KC - 1))
        xsq_t = []
        for kt in range(KC):
            xq = p_act.tile([128, T], FP16, tag="xq", bufs=4, name=f"xq{kt}")
            nc.scalar.activation(xq[:], xs[:, kt, :, :], AF.Square,
                                 bias=gr[:, G_CV + kt:G_CV + kt + 1])
            xsq_t.append(xq)
        for kt in range(KC):
            nc.tensor.matmul(st1[32:33, :], onesk[:], xsq_t[kt][:],
                             start=(kt == 0), stop=False)

        # repack to [128, 32], p = b*64 + h*8 + lhi, free = llo (l=lhi*32+llo)
        tt = p_sm.tile([128, FP], F32, tag="tt")
        for b in range(B):
            nc.sync.dma_start(tt[b * 64:(b + 1) * 64, :],
                              tt_sb[64:72, b * LC:(b + 1) * LC])

        def trickle(dep, nm):
            tkr = p_sm.tile([128, 8], F32R, tag="tkr", name=f"tkr{nm}")
            nc.gpsimd.tensor_copy(tkr[:], dep[:, 0:8])
            pw = ps_xa.tile([128, T], F32, tag="xa", name=f"trw{nm}")
            nc.tensor.matmul(pw[0:8, :], tkr[:], wtile[:], start=True, stop=True)

        # ---- f via Estrin on monomial coeffs (t2/t4 on scalar engine) ----
        t2 = p_sm.tile([128, FP], F32, tag="t2")
        nc.scalar.activation(t2[:], tt[:], AF.Square)
        t4 = p_sm.tile([128, FP], F32, tag="t4")
        nc.scalar.activation(t4[:], t2[:], AF.Square)
        dcol = lambda k: gr[:, G_CBB + k:G_CBB + k + 1]
        P_t = []
        for j in range(6):
            Pj = p_sm.tile([128, FP], F32, tag=f"P{j}", name=f"P{j}")
            nc.vector.tensor_scalar(Pj[:], tt[:], dcol(2 * j + 1), dcol(2 * j),
                                    op0=OP.mult, op1=OP.add)
            P_t.append(Pj)
        Q_t = []
        for i in range(3):
            cm = p_sm.tile([128, FP], F32, tag=f"c{i}", name=f"c{i}")
            nc.vector.tensor_mul(cm[:], t2[:], P_t[2 * i + 1][:])
            Qi = p_sm.tile([128, FP], F32, tag=f"Q{i}", name=f"Q{i}")
            nc.vector.tensor_add(Qi[:], P_t[2 * i][:], cm[:])
            Q_t.append(Qi)
            if i == 0:
                trickle(Qi, "q0")
        u1 = p_sm.tile([128, FP], F32, tag="u1")
        nc.vector.tensor_mul(u1[:], t4[:], Q_t[2][:])
        u2 = p_sm.tile([128, FP], F32, tag="u2")
        nc.vector.tensor_add(u2[:], Q_t[1][:], u1[:])
        trickle(u2, "u2")
        v1 = p_sm.tile([128, FP], F32, tag="v1")
        nc.vector.tensor_mul(v1[:], t4[:], u2[:])
        wp16 = p_sm.tile([128, FP], FP16, tag="wp16")
        nc.vector.tensor_add(wp16[:], Q_t[0][:], v1[:])

        # ---- unpack w to [H, T]; attention apply + LN1 stats fixes -------
        wH = p_sm.tile([72, T], FP16, tag="wH")
        for b in range(B):
            nc.sync.dma_start(wH[64:72, b * LC:(b + 1) * LC],
                              wp16[b * 64:(b + 1) * 64, :])
        # sum(y)/C += sum_h (uvs_h/C) w_h
        nc.tensor.matmul(st1[0:1, :], uvs_col, wH[64:72, :], start=False, stop=True)
        y_t = []
        for kt in range(KC):
            xa = ps_xa.tile([128, T], F32, tag="xa", name=f"xa{kt}")
            nc.tensor.matmul(xa[:], uvh[64:72, kt * 128:(kt + 1) * 128],
                             wH[64:72, :], start=True, stop=True)
            yk = p_act.tile([128, T], FP16, tag="y", bufs=4, name=f"y{kt}")
            nc.vector.scalar_tensor_tensor(
                out=yk[:], in0=xa[:], scalar=gr[:, G_CV + kt:G_CV + kt + 1],
                in1=xs[:, kt, :, :], op0=OP.add, op1=OP.add)
            y_t.append(yk)
        # sum(y^2)/C += (2/C) sum_h w_h [ (P_h + Pc_h) + (uvsq_h/2) w_h ]
        u8 = p_sm.tile([72, T], F32, tag="u8")
        nc.vector.tensor_scalar(u8[64:72, :], st1[64:72, :],
                                gr[64:72, G_PC:G_PC + 1], None, op0=OP.add)
        v8 = p_sm.tile([72, T], F32, tag="v8")
        nc.vector.scalar_tensor_tensor(out=v8[64:72, :], in0=wH[64:72, :],
                                       scalar=gr[64:72, G_UVQ2:G_UVQ2 + 1],
                                       in1=u8[64:72, :], op0=OP.mult, op1=OP.add)
        z8 = p_sm.tile([72, T], FP16, tag="z8")
        nc.vector.tensor_mul(z8[64:72, :], v8[64:72, :], wH[64:72, :])
        nc.tensor.matmul(st1[32:33, :], twoC_col, z8[64:72, :], start=False, stop=True)

        def ln_rows(stA, stB, ph, cvs=None, want_mu=False):
            """mean/meansq rows -> (mu, rstd) rows [1, T]."""
            musq = p_sm.tile([1, T], F32, tag="lnr", bufs=6, name=f"musq{ph}")
            nc.scalar.activation(musq[:], stA, AF.Square,
                                 bias=cvs if cvs is not None else 0.0)
            var = p_sm.tile([1, T], F32, tag="lnr", bufs=6, name=f"var{ph}")
            nc.vector.tensor_sub(var[:], stB, musq[:])
            std = p_sm.tile([1, T], F32R, tag="lnr", bufs=6, name=f"std{ph}")
            nc.scalar.activation(std[:], var[:], AF.Sqrt, bias=eps_col[:])
            pwln = ps_xa.tile([128, T], F32, tag="xa", name=f"pwln{ph}")
            nc.tensor.matmul(pwln[0:8, :], ones128[0:1, 0:8], std[:],
                             start=True, stop=True)
            rstd_f = p_sm.tile([1, T], F32, tag="rstdf", bufs=2, name=f"rstdf{ph}")
            nc.vector.reciprocal_approx_fast(rstd_f[:], std[:].bitcast(F32))
            rstd = p_sm.tile([1, T], F32R, tag="rstd", bufs=2, name=f"rstd{ph}")
            nc.vector.tensor_copy(rstd[:], rstd_f[:])
            mu = None
            if want_mu:
                mu = p_sm.tile([1, T], F32R, tag="mu", bufs=2, name=f"mu{ph}")
                if cvs is not None:
                    nc.vector.tensor_scalar(mu[:], stA, cvs, None, op0=OP.add)
                else:
                    nc.vector.tensor_copy(mu[:], stA)
            return mu, rstd

        # ---- LN1 apply -> x' = g1.(y - mu1).rstd1 ------------------------
        mu1, rstd1 = ln_rows(st1[0:1, :], st1[32:33, :], "a",
                             cvs=gr[0:1, G_CVS:G_CVS + 1], want_mu=True)
        mu1b = ps_xa.tile([128, T], F32, tag="xa", name="mu1b")
        nc.tensor.matmul(mu1b[:], ones128[:], mu1[:], start=True, stop=True)
        r1b = ps_xa.tile([128, T], F32, tag="xa", name="r1b")
        nc.tensor.matmul(r1b[:], ones128[:], rstd1[:], start=True, stop=True)
        x_t = []
        for kt in range(KC):
            yc = p_act.tile([128, T], FP16, tag="tx", bufs=2, name=f"yc{kt}")
            nc.vector.tensor_sub(yc[:], y_t[kt][:], mu1b[:])
            xo = p_act.tile([128, T], FP16, tag="x", bufs=4, name=f"x{kt}")
            nc.vector.scalar_tensor_tensor(
                out=xo[:], in0=yc[:], scalar=gr[:, G_G1 + kt:G_G1 + kt + 1],
                in1=r1b[:], op0=OP.mult, op1=OP.mult)
            x_t.append(xo)

        # ---- FFN1: h = relu(W1 @ x' + b1') -------------------------------
        h_t = []
        for mt in range(KH):
            pf = ps_mm.tile([128, T], F32, tag="mm", name=f"pf1{mt}")
            for kt in range(KC):
                nc.tensor.matmul(pf[:], w1s[:, kt, mt * 128:(mt + 1) * 128],
                                 x_t[kt][:], start=(kt == 0), stop=(kt == KC - 1))
            hm = p_act.tile([128, T], FP16, tag="h", bufs=KH, name=f"h{mt}")
            nc.scalar.activation(hm[:], pf[:], AF.Relu,
                                 bias=gr[:, G_B1 + mt:G_B1 + mt + 1])
            h_t.append(hm)

        # ---- FFN2 + residual -> y2 = x' + W2@h + b2'' --------------------
        y2_t = []
        st2 = ps_st.tile([33, T], F32, tag="st", name="st2")
        sq_t = []
        for mt in range(KC):
            pf = ps_mm.tile([128, T], F32, tag="mm", name=f"pf2{mt}")
            for kh in range(KH):
                nc.tensor.matmul(pf[:], w2s[:, kh, mt * 128:(mt + 1) * 128],
                                 h_t[kh][:], start=(kh == 0), stop=(kh == KH - 1))
            y2 = p_act.tile([128, T], FP16, tag="y2", bufs=4, name=f"y2{mt}")
            nc.vector.scalar_tensor_tensor(
                out=y2[:], in0=x_t[mt][:], scalar=gr[:, G_B2 + mt:G_B2 + mt + 1],
                in1=pf[:], op0=OP.add, op1=OP.add)
            y2_t.append(y2)
            nc.tensor.matmul(st2[0:1, :], onesk[:], y2[:],
                             start=(mt == 0), stop=(mt == KC - 1))
            sq = p_act.tile([128, T], FP16, tag="sq", bufs=4, name=f"sqb{mt}")
            nc.scalar.activation(sq[:], y2[:], AF.Square)
            sq_t.append(sq)

        # ---- LN2 folded into output projection ---------------------------
        # out = (Wo'@y2 + (-s2)(x)mu2) .* bcast(rstd2) + bo'
        for mt in range(KC):
            nc.tensor.matmul(st2[32:33, :], onesk[:], sq_t[mt][:],
                             start=(mt == 0), stop=(mt == KC - 1))
        po_t = [ps_mm.tile([128, T], F32, tag="mm", name=f"po{mt}")
                for mt in range(KC)]
        for kt in range(KC):
            for mt in range(KC):
                nc.tensor.matmul(po_t[mt][:],
                                 wos[:, kt, mt * 128:(mt + 1) * 128],
                                 y2_t[kt][:], start=(kt == 0), stop=False)
        mu2, rstd2 = ln_rows(st2[0:1, :], st2[32:33, :], "b", want_mu=True)
        rb_ps = ps_xa.tile([128, T], F32, tag="xa", name="rb")
        nc.tensor.matmul(rb_ps[:], ones128[:], rstd2[:], start=True, stop=True)
        rb_sb = p_sm.tile([128, T], F32, tag="rbs")
        nc.vector.tensor_copy(rb_sb[:], rb_ps[:])
        for mt in range(KC):
            nc.tensor.matmul(po_t[mt][:],
                             rowsr[0:1, RW_NS2 + mt * 128:RW_NS2 + (mt + 1) * 128],
                             mu2[:], start=False, stop=True)
        om_h = [p_act.tile([128, 2, T], FP16, tag="om", bufs=2, name=f"om{q}")
                for q in range(2)]
        for mt in range(KC):
            vm = p_act.tile([128, T], F32, tag="vm", bufs=2, name=f"vm{mt}")
            nc.vector.tensor_mul(vm[:], po_t[mt][:], rb_sb[:])
            om = om_h[mt // 2]
            nc.scalar.activation(om[:, mt % 2, :], vm[:], AF.Identity,
                                 bias=gr[:, G_BO + mt:G_BO + mt + 1])
            if mt % 2 == 1:
                seng = nc.sync if mt == 1 else nc.scalar
                seng.dma_start(out_sl[:, mt - 1:mt + 1, :], om[:])

    nc.compile()
    return nc


def _host_pack(inputs):
    f32 = lambda x: np.asarray(x, dtype=np.float32)
    Wq, Wk, Wv, Wo = (f32(inputs[k]) for k in ("Wq", "Wk", "Wv", "Wo"))
    W1, W2 = f32(inputs["W1"]), f32(inputs["W2"])
    Wg = f32(inputs["Wg"])[:, 0]
    bg, bq, bv, b1, b2, bo = (f32(inputs[k]) for k in ("bg", "bq", "bv", "b1", "b2", "bo"))
    g1, be1, g2, be2 = (f32(inputs[k]) for k in ("g1", "beta1", "g2", "beta2"))
    expv = np.asarray(inputs["exp"], dtype=np.float64)

    u_k = Wk @ Wg
    u_v = Wv @ Wg
    c_v = Wv @ bg + bv
    M = np.zeros((C, H), np.float32)
    a0 = np.zeros(H, np.float32)
    for h in range(H):
        ukh = u_k[h * D:(h + 1) * D]
        M[:, h] = Wq[h * D:(h + 1) * D, :].T @ ukh
        a0[h] = bq[h * D:(h + 1) * D] @ ukh
    Mp = M * (SCALE / SCAL)
    a0p = a0 * (SCALE / SCAL)
    uvH = np.zeros((H, C), np.float32)
    for h in range(H):
        uvH[h, h * D:(h + 1) * D] = u_v[h * D:(h + 1) * D]
    Wop = Wo * g2[None, :]
    bop = bo + Wo @ be2
    s2 = Wop.sum(1)
    b1p = b1 + W1 @ be1
    b2p = b2 + be1

    # f_b fit from the tiny `exp` input -> monomial coeffs (Estrin on device)
    import numpy.polynomial.chebyshev as Ch
    lnz = np.zeros((MN, B))
    for b in range(B):
        lnz[:, b] = np.log(np.exp(_XN[:, None] * expv[b][None, :]).sum(1))
    ck = _DM @ lnz                                           # [KD, B]
    dmono = np.zeros((KD, B))
    for b in range(B):
        dd = Ch.cheb2poly(ck[:, b])
        dmono[:len(dd), b] = dd

    gr = np.zeros((128, GN), np.float32)
    gr[0:64, G_CBB:G_CBB + KD] = dmono[:, 0]
    gr[64:128, G_CBB:G_CBB + KD] = dmono[:, 1]
    for kt in range(KC):
        gr[:, G_CV + kt] = c_v[kt * 128:(kt + 1) * 128]
        gr[:, G_B2 + kt] = b2p[kt * 128:(kt + 1) * 128]
        gr[:, G_G1 + kt] = g1[kt * 128:(kt + 1) * 128]
        gr[:, G_BO + kt] = bop[kt * 128:(kt + 1) * 128]
    for mt in range(KH):
        gr[:, G_B1 + mt] = b1p[mt * 128:(mt + 1) * 128]
    gr[64:72, G_A0] = a0p
    gr[0, G_CVS] = c_v.sum() / C
    uv_cv = uvH * c_v[None, :]                               # [H, C]
    gr[64:72, G_PC] = uv_cv.sum(1)
    gr[64:72, G_UVQ2] = (uvH ** 2).sum(1) / 2.0
    tmp = np.zeros((128, 2), np.float32)
    tv = tmp.view(np.float16)
    tv[64:72, 0] = (uvH.sum(1) / C).astype(np.float16)
    tv[64:72, 1] = np.float16(2.0 / C)
    gr[:, G_F16:G_F16 + 2] = tmp
    tmp = np.zeros((128, 16), np.float32)
    tv = tmp.view(np.float16)                                # [128, 32]
    for kt in range(KC):
        tv[:, kt * 8:(kt + 1) * 8] = uvH[:, kt * 128:(kt + 1) * 128].T
    gr[:, G_UVT:G_UVT + 16] = tmp
    tmp = np.zeros((128, 16), np.float32)
    tv = tmp.view(np.float16)
    for kt in range(KC):
        tv[:, kt * 8:(kt + 1) * 8] = Mp[kt * 128:(kt + 1) * 128, :]
    gr[:, G_M16:G_M16 + 16] = tmp

    rowsv = np.zeros((1, RW_NCOL), np.float32)
    rowsv[0, RW_NS2:RW_NS2 + C] = -s2

    f16t = lambda x: np.ascontiguousarray(x.T, dtype=np.float16)
    return {
        "grid": gr,
        "rowsv": rowsv,
        "uvha": np.asarray(uvH, dtype=np.float16),
        "w1a": f16t(W1),
        "w2a": f16t(W2),
        "woa": f16t(Wop),
    }


def kernel(**inputs):
    global _CACHE, LAST_RESULTS
    if _CACHE is None:
        _CACHE = _build()
    nc = _CACHE

    base = _host_pack(inputs)
    seq = np.asarray(inputs["seq"], dtype=np.float16)
    # pre-pack seq to device layout [128, ktp, kt2, b, l]
    seq4 = seq.reshape(B, 2, 2, 128, L).transpose(3, 1, 2, 0, 4)  # [128,ktp,kt2,B,L]
    in_maps = []
    for c in range(NCORES):
        m = dict(base)
        m["seqp"] = np.ascontiguousarray(
            seq4[:, :, :, :, c * LC:(c + 1) * LC]).reshape(128, 2, 2 * B * LC)
        in_maps.append(m)

    res = run_bass_kernel_spmd(nc, in_maps, list(range(NCORES)), trace=TRACE,
                               **TRACE_KW)
    LAST_RESULTS = res
    out = np.empty((B, C, L), np.float32)
    for c in range(NCORES):
        o = res.results[c]["out_sl"].astype(np.float32)      # [128, KC, B*LC]
        o = o.reshape(128, KC, B, LC).transpose(2, 1, 0, 3)  # [B, KC, 128, LC]
        out[:, :, c * LC:(c + 1) * LC] = o.reshape(B, C, LC)
    return out
